# revision 14
# baseline (speedup 1.0000x reference)
"""Trainium2 Bass kernel: FADEv4 retrieval-kNN head (nn_FADEv4_7026566496861).

Math (per image n):
    cls  = l2norm(mean_s(x_support_cls[n]))          # [1,D]
    q    = l2norm(x_query[n])                        # [Tq,D]
    s    = l2norm(x_support[n])                      # [Ts,D]
    sim  = q @ s.T                                   # [Tq,Ts]
    dmin = 1 - max_ts(sim); idx = argmax_ts(sim)
    pred = sigmoid(q@W1 + s[idx]@W2 + cls@W3 + b)
    out0 = (pred*dmin).reshape(N,1,37,37); out1 = pred.reshape(N,1,37,37)

Sharding: data-parallel over N=16 images -> 8 cores x 2 images, no collectives.

Design:
  * Normalize on Act engine (Square+accum -> sqrt -> recip -> mul to bf16),
    then transpose each [tok,768] chunk to [768,tok] layout with the DMA
    XBAR transpose (dma_start(..., transpose=True)) - no PE transposes, no
    PSUM staging, no PSUM->SBUF copy pass.
  * Full sT [128,6,5504] bf16 resident; main loop is quad-outer (3 column
    quads of 2048), m-inner; each (m,Q) accumulates 6 K-chunks into a
    4-bank [128,2048] f32 PSUM quad. W1 appended as an extra support
    column (-> p1 = q@W1), W2 as an extra query column (-> p2 = s@W2).
  * Single-pass max+argmax on the DVE via a custom fused op (PACKMAX):
      body  = ((sim + (2+2^14)) - 2^14) * 2^22 + iota   ; accum = MAX
    i.e. quantize sim to a 2^-9 grid (round-to-nearest via the add-big
    trick), scale to a multiple of 2^13, and add (8191 - global_index).
    The running MAX of the packed value yields both the (quantized) max
    and the argmax (first occurrence on ties), decoded with a few small
    vector ops per image. Replaces the max8 + find_index8 double scan.
  * p2 = s@W2 staged to DRAM; gathered per query row by indirect DMA.
"""

import numpy as np

from contextlib import ExitStack

import concourse.bass as bass
import concourse.mybir as mybir
import concourse.tile as tile
from concourse import bacc
from concourse.bass import ds, ts, IndirectOffsetOnAxis
from concourse.bass_utils import run_bass_kernel_spmd

F32 = mybir.dt.float32
BF16 = mybir.dt.bfloat16
U32 = mybir.dt.uint32
AX = mybir.AxisListType
OP = mybir.AluOpType
ACTF = mybir.ActivationFunctionType

# ---------------- problem dims ----------------
N_FULL, TQ, TS, S, D = 16, 1369, 5476, 4, 768
SIDE = 37
KC = D // 128                 # 6 contraction chunks
N_CORES = 8
PER_CORE = N_FULL // N_CORES

MB = 11                       # query 128-row blocks (last has 89 real rows)
SBANKS = 11                   # sT 512-col banks (cols 0..5631)
QBANKS = 3                    # qT 512-col banks (cols 0..1535)
W1BANK, W1LOC = 10, 368       # W1 support col: global 5488
W2BANK, W2LOC = 2, 352        # W2 query col: global 1376

# quads: (base, [bank widths], pack width); pack excludes pad cols + W1
QUADS = [
    (0, [512, 512, 512, 512], 2048),
    (2048, [512, 512, 512, 512], 2048),
    (4096, [512, 512, 384], TS - 4096),
]
P1LOC = 1392                  # p1 column local to quad 2 (1024 + 368)

# packmax constants: pack = RN(sim+2 @2^-9)*2^22 + (8191-idx)
PK_C0 = 2.0 + 16384.0
PK_C1 = 16384.0
PK_C2 = 4194304.0
BIG = 8388608.0               # 2^23 round-to-int bias

# ---------------- custom DVE op ----------------
from concourse.dve_spec import Spec, Src0, Src1, C0, C1, C2, maxx, lower
from concourse.dve_ops import DveOp, OPS
import concourse.dve_ops as _dvo
from concourse.dve_uop import DveOpSpec


def _packmax_ref(in0, in1, c0, c1, c2):
    x = in0.astype(np.float32)
    b = ((x + np.float32(c0)) - np.float32(c1)) * np.float32(c2)
    body = (b + in1.astype(np.float32)).astype(np.float32)
    acc = body.reshape(body.shape[0], -1).max(axis=-1, keepdims=True)
    return body, acc


def _register_packmax():
    for o in OPS:
        if o.name == "PACKMAX_ANT":
            return o
    spec = Spec(body=((Src0 + C0) - C1) * C2 + Src1, accum=maxx,
                reference=_packmax_ref)
    shas = {}
    for ver in ("v3", "v4"):
        dspec = DveOpSpec(name="PACKMAX_ANT", opcode=0,
                          uops=lower(spec, ver=ver), rd1_en=True)
        shas[ver] = dspec.sha(ver)
    op = DveOp("PACKMAX_ANT", spec, subdim=False, uops_sha=shas)
    OPS.append(op)
    _dvo.CUSTOM_DVE_SPECS[op.name] = op.spec
    _dvo._SUB_OPCODE_FOR_NAME[op.name] = _dvo._CUSTOM_DVE_ROW_BASE + len(OPS) - 1
    return op


PACKMAX = _register_packmax()


def _ceil16(x):
    return (x + 15) // 16 * 16


def build_program(per_core=PER_CORE):
    nc = bacc.Bacc("TRN2", target_bir_lowering=False, debug=False)
    x_query = nc.dram_tensor("x_query", [per_core, TQ, D], F32, kind="ExternalInput").ap()
    x_support = nc.dram_tensor("x_support", [per_core, TS, D], F32, kind="ExternalInput").ap()
    x_cls = nc.dram_tensor("x_support_cls", [per_core, S * D], F32, kind="ExternalInput").ap()
    w_head = nc.dram_tensor("W_head", [3 * D, 1], F32, kind="ExternalInput").ap()
    b_head = nc.dram_tensor("b_head", [1, 1], F32, kind="ExternalInput").ap()
    out0 = nc.dram_tensor("out0", [per_core, TQ], F32, kind="ExternalOutput").ap()
    out1 = nc.dram_tensor("out1", [per_core, TQ], F32, kind="ExternalOutput").ap()
    p2d_list = [nc.dram_tensor(f"p2d_{n}", [TS, 1], F32).ap() for n in range(per_core)]
    c3d_list = [nc.dram_tensor(f"c3d_{n}", [1, 1], F32).ap() for n in range(per_core)]

    with tile.TileContext(nc) as tc, ExitStack() as ctx:
        const_pool = ctx.enter_context(tc.tile_pool(name="const", bufs=1))
        st_pool = ctx.enter_context(tc.tile_pool(name="st", bufs=3))
        qt_pool = ctx.enter_context(tc.tile_pool(name="qt", bufs=2))
        img_pool = ctx.enter_context(tc.tile_pool(name="img", bufs=2))
        scratch = ctx.enter_context(tc.tile_pool(name="scr", bufs=4))
        cls_pool = ctx.enter_context(tc.tile_pool(name="cls", bufs=1))
        pk_pool = ctx.enter_context(tc.tile_pool(name="pk", bufs=1))
        psum_mm = ctx.enter_context(tc.tile_pool(name="pmm", bufs=2, space="PSUM"))

        # ---- constants ----
        w1s = const_pool.tile([128, KC], F32)
        w2s = const_pool.tile([128, KC], F32)
        w3 = const_pool.tile([1, D], F32)
        bh = const_pool.tile([1, 1], F32)
        for k in range(KC):
            nc.sync.dma_start(out=w1s[:, k:k + 1], in_=w_head[ds(128 * k, 128), :])
            nc.sync.dma_start(out=w2s[:, k:k + 1], in_=w_head[ds(D + 128 * k, 128), :])
        nc.sync.dma_start(out=w3[0:1, :], in_=w_head[ds(2 * D, D), :])
        nc.sync.dma_start(out=bh[:, :], in_=b_head[:, :])

        iof = const_pool.tile([128, 3 * 2048], F32)
        for Qi in range(3):
            iou = cls_pool.tile([128, 2048], U32, tag="iou")
            nc.gpsimd.iota(iou[:, :], pattern=[[-1, 2048]], base=8191 - 2048 * Qi,
                           channel_multiplier=0)
            nc.vector.tensor_copy(iof[:, ds(2048 * Qi, 2048)], iou[:, :])

        def build_chunks(n, src, dstT, bank0, tok0, ntok):
            """Normalize rows [tok0, tok0+ntok) of src[n]; write transposed
            bf16 into dstT (bank-major [128, B, KC, 512]), banks rel bank0."""
            done = 0
            while done < ntok:
                rows = min(128, ntok - done)
                r16 = _ceil16(rows)
                t0 = tok0 + done
                raw = scratch.tile([128, D], F32, tag="raw")
                if r16 > rows:
                    nc.vector.memset(raw[:r16, :], 0)
                nc.sync.dma_start(out=raw[:rows, :], in_=src[n][ds(t0, rows), :])
                sq = scratch.tile([128, D], BF16, tag="sq")
                nrm = scratch.tile([128, 3], F32, tag="nrm")
                nc.scalar.activation(sq[:r16, :], raw[:r16, :], ACTF.Square,
                                     accum_out=nrm[:r16, 0:1])
                nc.scalar.activation(nrm[:r16, 2:3], nrm[:r16, 0:1],
                                     ACTF.Abs_reciprocal_sqrt)
                s_nm = scratch.tile([128, D], BF16, tag="snm")
                nc.scalar.mul(s_nm[:r16, :], raw[:r16, :], nrm[:r16, 2:3])
                cb, ci = t0 // 512 - bank0, t0 % 512
                nc.sync.dma_start(out=dstT[:, cb, :, ds(ci, r16)],
                                  in_=s_nm[:r16, :], transpose=True)
                done += rows

        ims = [{} for _ in range(per_core)]

        # ================= phase 1: builds (all images) =================
        for n in range(per_core):
            im = ims[n]
            sQ = []
            for Qi, (base, bws, pkw) in enumerate(QUADS):
                t = st_pool.tile([128, 4, KC, 512], BF16, tag="sQ", name=f"sQ{n}_{Qi}")
                ntok = min(TS, base + 2048) - base
                build_chunks(n, x_support, t, base // 512, base, ntok)
                sQ.append(t)
                if Qi == 0:
                    qT = qt_pool.tile([128, QBANKS, KC, 512], BF16, tag="qT",
                                      name=f"qT{n}")
                    build_chunks(n, x_query, qT, 0, 0, TQ)
                    im["qT"] = qT
            im["sQ"] = sQ

            # cls head scalar: sum shots via accumulating SWDGE DMA
            clsum = cls_pool.tile([1, D], F32, tag="clsum")
            nc.gpsimd.dma_start(out=clsum[:, :], in_=x_cls[n][ds(0, D)])
            for sh in range(1, S):
                nc.gpsimd.dma_start(out=clsum[:, :],
                                    in_=x_cls[n][ds(sh * D, D)],
                                    accum_op=OP.add)
            sc3 = cls_pool.tile([1, D], F32, tag="sc3")
            ss3 = cls_pool.tile([1, 8], F32, tag="ss3")
            nc.vector.tensor_mul(sc3[:, :], clsum[:, :], clsum[:, :])
            nc.vector.tensor_reduce(out=ss3[:, 0:1], in_=sc3[:, :], axis=AX.X, op=OP.add)
            nc.vector.tensor_mul(sc3[:, :], clsum[:, :], w3[:, :])
            nc.vector.tensor_reduce(out=ss3[:, 1:2], in_=sc3[:, :], axis=AX.X, op=OP.add)
            nc.scalar.sqrt(ss3[:, 2:3], ss3[:, 0:1])
            nc.vector.reciprocal(ss3[:, 3:4], ss3[:, 2:3])
            nc.vector.tensor_mul(ss3[:, 4:5], ss3[:, 1:2], ss3[:, 3:4])
            nc.vector.tensor_add(ss3[:, 5:6], ss3[:, 4:5], bh[:, 0:1])
            nc.sync.dma_start(out=c3d_list[n][:, :], in_=ss3[0:1, 5:6])
            c3b = img_pool.tile([128, 1], F32, tag="c3b")
            nc.sync.dma_start(out=c3b[:, :], in_=c3d_list[n][:, :].to_broadcast((128, 1)))
            im["c3b"] = c3b

        # ================= phase 2: main + tail per image =================
        for n in range(per_core):
            im = ims[n]
            qT, sQ, c3b = im["qT"], im["sQ"], im["c3b"]
            for k in range(KC):
                nc.vector.tensor_copy(sQ[2][:, 2, k, W1LOC:W1LOC + 1],
                                      w1s[:, k:k + 1])
                nc.vector.tensor_copy(qT[:, W2BANK, k, W2LOC:W2LOC + 1],
                                      w2s[:, k:k + 1])
            Mc = img_pool.tile([128, MB, 3], F32, tag="Mc")
            p1_all = img_pool.tile([128, MB], F32, tag="p1")
            pscr = pk_pool.tile([128, 2048], F32, tag="pscr")

            for Qi, (base, bws, pkw) in enumerate(QUADS):
                for m in range(MB):
                    mcols = 128 if m < MB - 1 else (W2BANK * 512 + W2LOC + 1) - 128 * (MB - 1)
                    quad = psum_mm.tile([128, 2048], F32, tag="quad")
                    for k in range(KC):
                        for bi, nb in enumerate(bws):
                            nc.tensor.matmul(
                                quad[:mcols, ds(512 * bi, nb)],
                                lhsT=qT[:, m // 4, k, ds(128 * (m % 4), mcols)],
                                rhs=sQ[Qi][:, bi, k, :nb],
                                start=(k == 0), stop=(k == KC - 1),
                            )
                    nc.vector._custom_dve(
                        PACKMAX, out=pscr[:mcols, :pkw],
                        in0=quad[:mcols, :pkw],
                        in1=iof[:mcols, ds(2048 * Qi, pkw)],
                        s0=PK_C0, s1=PK_C1, imm2=PK_C2,
                        accum_out=Mc[:mcols, m, Qi:Qi + 1],
                    )
                    if m == MB - 1:
                        p2c = cls_pool.tile([1, 2048], F32, tag="p2c")
                        nc.vector.tensor_copy(p2c[0:1, :pkw], quad[96:97, :pkw])
                        nc.sync.dma_start(out=p2d_list[n][ds(base, pkw), 0],
                                          in_=p2c[0:1, :pkw])
                    if Qi == 2:
                        nc.vector.tensor_copy(p1_all[:mcols, m:m + 1],
                                              quad[:mcols, P1LOC:P1LOC + 1])

            # ---- combine: reduce quads, decode pack ----
            red = img_pool.tile([128, MB], F32, tag="red")
            nc.vector.tensor_reduce(out=red[:, :], in_=Mc[:, :, :], axis=AX.X,
                                    op=OP.max)
            dec = img_pool.tile([128, 6 * MB], F32, tag="dec")
            t_ = dec[:, 0 * MB:1 * MB]
            r_ = dec[:, 1 * MB:2 * MB]
            d_ = dec[:, 2 * MB:3 * MB]
            ng = dec[:, 3 * MB:4 * MB]
            u_ = dec[:, 4 * MB:5 * MB]
            k_ = dec[:, 5 * MB:6 * MB]
            nc.vector.tensor_scalar_mul(t_, red[:, :], 1.0 / 8192.0)
            nc.scalar.activation(r_, t_, ACTF.Copy, bias=BIG)
            nc.vector.tensor_scalar_add(r_, r_, -BIG)
            nc.vector.tensor_sub(d_, t_, r_)
            nc.vector.tensor_scalar(ng, d_, 0.0, None, op0=OP.is_lt)
            nc.vector.tensor_add(u_, d_, ng)
            nc.vector.tensor_scalar_mul(u_, u_, 8192.0)
            nc.vector.tensor_sub(k_, r_, ng)
            gidxf = img_pool.tile([128, MB], F32, tag="gidxf")
            nc.scalar.activation(gidxf[:, :], u_, ACTF.Copy, bias=8191.0, scale=-1.0)
            gidx = img_pool.tile([128, MB], U32, tag="gidx")
            nc.vector.tensor_copy(gidx[:, :], gidxf[:, :])
            dmin_all = img_pool.tile([128, MB], F32, tag="dmin")
            nc.scalar.activation(dmin_all[:, :], k_, ACTF.Copy, bias=3.0,
                                 scale=-1.0 / 512.0)

            # ---- p2 gather + head ----
            p2g = img_pool.tile([128, MB], F32, tag="p2g")
            for m in range(MB):
                nc.gpsimd.indirect_dma_start(
                    out=p2g[:, m:m + 1], out_offset=None, in_=p2d_list[n][:, :],
                    in_offset=IndirectOffsetOnAxis(ap=gidx[:, m:m + 1], axis=0),
                )
            lg = img_pool.tile([128, MB], F32, tag="lg")
            nc.vector.tensor_add(lg[:, :], p1_all[:, :], p2g[:, :])
            pred = img_pool.tile([128, MB], F32, tag="pred")
            nc.scalar.activation(pred[:, :], lg[:, :], ACTF.Sigmoid, bias=c3b[:, :])
            o0 = img_pool.tile([128, MB], F32, tag="o0")
            nc.vector.tensor_mul(o0[:, :], pred[:, :], dmin_all[:, :])
            for m in range(MB):
                mreal = 128 if m < MB - 1 else TQ - 128 * (MB - 1)
                nc.sync.dma_start(out=out1[n, ds(m * 128, mreal)],
                                  in_=pred[:mreal, m:m + 1])
                nc.sync.dma_start(out=out0[n, ds(m * 128, mreal)],
                                  in_=o0[:mreal, m:m + 1])

    nc.compile()
    return nc


_CACHED = {}


def _get_program(per_core=PER_CORE):
    if per_core not in _CACHED:
        _CACHED[per_core] = build_program(per_core)
    return _CACHED[per_core]


def run(inputs, trace=False, per_core=PER_CORE):
    nc = _get_program(per_core)
    n_cores = N_FULL // per_core
    xq = np.ascontiguousarray(inputs["x_query"], dtype=np.float32)
    xs = np.ascontiguousarray(inputs["x_support"], dtype=np.float32)
    xc = np.ascontiguousarray(inputs["x_support_cls"], dtype=np.float32).reshape(
        N_FULL, S * D
    )
    wh = np.ascontiguousarray(inputs["W_head"], dtype=np.float32).reshape(3 * D, 1)
    bhv = np.ascontiguousarray(inputs["b_head"], dtype=np.float32).reshape(1, 1)
    in_maps = []
    for c in range(n_cores):
        sl = slice(c * per_core, (c + 1) * per_core)
        in_maps.append({
            "x_query": xq[sl], "x_support": xs[sl], "x_support_cls": xc[sl],
            "W_head": wh, "b_head": bhv,
        })
    res = run_bass_kernel_spmd(nc, in_maps, list(range(n_cores)), trace=trace)
    o0 = np.concatenate([res.results[c]["out0"] for c in range(n_cores)], axis=0)
    o1 = np.concatenate([res.results[c]["out1"] for c in range(n_cores)], axis=0)
    o0 = o0.reshape(N_FULL, 1, SIDE, SIDE).astype(np.float32)
    o1 = o1.reshape(N_FULL, 1, SIDE, SIDE).astype(np.float32)
    return (o0, o1), res


def kernel(**inputs):
    (o0, o1), _ = run(inputs, trace=False)
    return o0, o1


# revision 15
# speedup vs baseline: 1.0299x; 1.0299x over previous
"""Trainium2 Bass kernel: FADEv4 retrieval-kNN head (nn_FADEv4_7026566496861).

Math (per image n):
    cls  = l2norm(mean_s(x_support_cls[n]))          # [1,D]
    q    = l2norm(x_query[n])                        # [Tq,D]
    s    = l2norm(x_support[n])                      # [Ts,D]
    sim  = q @ s.T                                   # [Tq,Ts]
    dmin = 1 - max_ts(sim); idx = argmax_ts(sim)
    pred = sigmoid(q@W1 + s[idx]@W2 + cls@W3 + b)
    out0 = (pred*dmin).reshape(N,1,37,37); out1 = pred.reshape(N,1,37,37)

Sharding: data-parallel over N=16 images -> 8 cores x 2 images, no collectives.

Design:
  * Normalize on Act engine (Square+accum -> sqrt -> recip -> mul to bf16),
    then transpose each [tok,768] chunk to [768,tok] layout with the DMA
    XBAR transpose (dma_start(..., transpose=True)) - no PE transposes, no
    PSUM staging, no PSUM->SBUF copy pass.
  * Full sT [128,6,5504] bf16 resident; main loop is quad-outer (3 column
    quads of 2048), m-inner; each (m,Q) accumulates 6 K-chunks into a
    4-bank [128,2048] f32 PSUM quad. W1 appended as an extra support
    column (-> p1 = q@W1), W2 as an extra query column (-> p2 = s@W2).
  * Single-pass max+argmax on the DVE via a custom fused op (PACKMAX):
      body  = ((sim + (2+2^14)) - 2^14) * 2^22 + iota   ; accum = MAX
    i.e. quantize sim to a 2^-9 grid (round-to-nearest via the add-big
    trick), scale to a multiple of 2^13, and add (8191 - global_index).
    The running MAX of the packed value yields both the (quantized) max
    and the argmax (first occurrence on ties), decoded with a few small
    vector ops per image. Replaces the max8 + find_index8 double scan.
  * p2 = s@W2 staged to DRAM; gathered per query row by indirect DMA.
"""

import numpy as np

from contextlib import ExitStack

import concourse.bass as bass
import concourse.mybir as mybir
import concourse.tile as tile
from concourse import bacc
from concourse.bass import ds, ts, IndirectOffsetOnAxis
from concourse.bass_utils import run_bass_kernel_spmd

F32 = mybir.dt.float32
BF16 = mybir.dt.bfloat16
U32 = mybir.dt.uint32
AX = mybir.AxisListType
OP = mybir.AluOpType
ACTF = mybir.ActivationFunctionType

# ---------------- problem dims ----------------
N_FULL, TQ, TS, S, D = 16, 1369, 5476, 4, 768
SIDE = 37
KC = D // 128                 # 6 contraction chunks
N_CORES = 8
PER_CORE = N_FULL // N_CORES

MB = 11                       # query 128-row blocks (last has 89 real rows)
SBANKS = 11                   # sT 512-col banks (cols 0..5631)
QBANKS = 3                    # qT 512-col banks (cols 0..1535)
W1BANK, W1LOC = 10, 368       # W1 support col: global 5488
W2BANK, W2LOC = 2, 352        # W2 query col: global 1376

# quads: (base, [bank widths], pack width); pack excludes pad cols + W1
QUADS = [
    (0, [512, 512, 512, 512], 2048),
    (2048, [512, 512, 512, 512], 2048),
    (4096, [512, 512, 384], TS - 4096),
]
P1LOC = 1392                  # p1 column local to quad 2 (1024 + 368)

# packmax constants: pack = RN(sim+2 @2^-9)*2^22 + (8191-idx)
PK_C0 = 2.0 + 16384.0
PK_C1 = 16384.0
PK_C2 = 4194304.0
BIG = 8388608.0               # 2^23 round-to-int bias

# ---------------- custom DVE op ----------------
from concourse.dve_spec import Spec, Src0, Src1, C0, C1, C2, maxx, lower
from concourse.dve_ops import DveOp, OPS
import concourse.dve_ops as _dvo
from concourse.dve_uop import DveOpSpec


def _packmax_ref(in0, in1, c0, c1, c2):
    x = in0.astype(np.float32)
    b = ((x + np.float32(c0)) - np.float32(c1)) * np.float32(c2)
    body = (b + in1.astype(np.float32)).astype(np.float32)
    acc = body.reshape(body.shape[0], -1).max(axis=-1, keepdims=True)
    return body, acc


def _register_packmax():
    for o in OPS:
        if o.name == "PACKMAX_ANT":
            return o
    spec = Spec(body=((Src0 + C0) - C1) * C2 + Src1, accum=maxx,
                reference=_packmax_ref)
    shas = {}
    for ver in ("v3", "v4"):
        dspec = DveOpSpec(name="PACKMAX_ANT", opcode=0,
                          uops=lower(spec, ver=ver), rd1_en=True)
        shas[ver] = dspec.sha(ver)
    op = DveOp("PACKMAX_ANT", spec, subdim=False, uops_sha=shas)
    OPS.append(op)
    _dvo.CUSTOM_DVE_SPECS[op.name] = op.spec
    _dvo._SUB_OPCODE_FOR_NAME[op.name] = _dvo._CUSTOM_DVE_ROW_BASE + len(OPS) - 1
    return op


PACKMAX = _register_packmax()


def _ceil16(x):
    return (x + 15) // 16 * 16


def build_program(per_core=PER_CORE):
    nc = bacc.Bacc("TRN2", target_bir_lowering=False, debug=False)
    x_query = nc.dram_tensor("x_query", [per_core, TQ, D], F32, kind="ExternalInput").ap()
    x_support = nc.dram_tensor("x_support", [per_core, TS, D], F32, kind="ExternalInput").ap()
    x_cls = nc.dram_tensor("x_support_cls", [per_core, S * D], F32, kind="ExternalInput").ap()
    w_head = nc.dram_tensor("W_head", [3 * D, 1], F32, kind="ExternalInput").ap()
    b_head = nc.dram_tensor("b_head", [1, 1], F32, kind="ExternalInput").ap()
    out0 = nc.dram_tensor("out0", [per_core, TQ], F32, kind="ExternalOutput").ap()
    out1 = nc.dram_tensor("out1", [per_core, TQ], F32, kind="ExternalOutput").ap()
    p2d_list = [nc.dram_tensor(f"p2d_{n}", [TS, 1], F32).ap() for n in range(per_core)]
    c3d_list = [nc.dram_tensor(f"c3d_{n}", [1, 1], F32).ap() for n in range(per_core)]

    with tile.TileContext(nc) as tc, ExitStack() as ctx:
        const_pool = ctx.enter_context(tc.tile_pool(name="const", bufs=1))
        st_pool = ctx.enter_context(tc.tile_pool(name="st", bufs=3))
        qt_pool = ctx.enter_context(tc.tile_pool(name="qt", bufs=2))
        img_pool = ctx.enter_context(tc.tile_pool(name="img", bufs=2))
        scratch = ctx.enter_context(tc.tile_pool(name="scr", bufs=4))
        cls_pool = ctx.enter_context(tc.tile_pool(name="cls", bufs=1))
        pk_pool = ctx.enter_context(tc.tile_pool(name="pk", bufs=1))
        psum_mm = ctx.enter_context(tc.tile_pool(name="pmm", bufs=2, space="PSUM"))

        # ---- constants ----
        w1s = const_pool.tile([128, KC], F32)
        w2s = const_pool.tile([128, KC], F32)
        w3 = const_pool.tile([1, D], F32)
        bh = const_pool.tile([1, 1], F32)
        for k in range(KC):
            nc.sync.dma_start(out=w1s[:, k:k + 1], in_=w_head[ds(128 * k, 128), :])
            nc.sync.dma_start(out=w2s[:, k:k + 1], in_=w_head[ds(D + 128 * k, 128), :])
        nc.sync.dma_start(out=w3[0:1, :], in_=w_head[ds(2 * D, D), :])
        nc.sync.dma_start(out=bh[:, :], in_=b_head[:, :])

        iof = const_pool.tile([128, 3 * 2048], F32)
        for Qi in range(3):
            iou = cls_pool.tile([128, 2048], U32, tag="iou")
            nc.gpsimd.iota(iou[:, :], pattern=[[-1, 2048]], base=8191 - 2048 * Qi,
                           channel_multiplier=0)
            nc.vector.tensor_copy(iof[:, ds(2048 * Qi, 2048)], iou[:, :])

        def build_chunks(n, src, dstT, bank0, tok0, ntok):
            """Normalize rows [tok0, tok0+ntok) of src[n]; write transposed
            bf16 into dstT (bank-major [128, B, KC, 512]), banks rel bank0."""
            done = 0
            while done < ntok:
                rows = min(128, ntok - done)
                r16 = _ceil16(rows)
                t0 = tok0 + done
                raw = scratch.tile([128, D], F32, tag="raw")
                if r16 > rows:
                    nc.vector.memset(raw[:r16, :], 0)
                nc.sync.dma_start(out=raw[:rows, :], in_=src[n][ds(t0, rows), :])
                sq = scratch.tile([128, D], BF16, tag="sq")
                nrm = scratch.tile([128, 3], F32, tag="nrm")
                nc.scalar.activation(sq[:r16, :], raw[:r16, :], ACTF.Square,
                                     accum_out=nrm[:r16, 0:1])
                nc.scalar.activation(nrm[:r16, 1:2], nrm[:r16, 0:1], ACTF.Sqrt)
                s_nm = scratch.tile([128, D], BF16, tag="snm")
                nc.gpsimd.normalize_recip(s_nm[:r16, :], raw[:r16, :],
                                          nrm[:r16, 1:2])
                cb, ci = t0 // 512 - bank0, t0 % 512
                nc.sync.dma_start(out=dstT[:, cb, :, ds(ci, r16)],
                                  in_=s_nm[:r16, :], transpose=True)
                done += rows

        ims = [{} for _ in range(per_core)]

        # ================= phase 1: builds (all images) =================
        for n in range(per_core):
            im = ims[n]
            sQ = []
            for Qi, (base, bws, pkw) in enumerate(QUADS):
                t = st_pool.tile([128, 4, KC, 512], BF16, tag="sQ", name=f"sQ{n}_{Qi}")
                ntok = min(TS, base + 2048) - base
                build_chunks(n, x_support, t, base // 512, base, ntok)
                sQ.append(t)
                if Qi == 0:
                    qT = qt_pool.tile([128, QBANKS, KC, 512], BF16, tag="qT",
                                      name=f"qT{n}")
                    build_chunks(n, x_query, qT, 0, 0, TQ)
                    im["qT"] = qT
            im["sQ"] = sQ

            # cls head scalar: sum shots via accumulating SWDGE DMA
            clsum = cls_pool.tile([1, D], F32, tag="clsum")
            nc.gpsimd.dma_start(out=clsum[:, :], in_=x_cls[n][ds(0, D)])
            for sh in range(1, S):
                nc.gpsimd.dma_start(out=clsum[:, :],
                                    in_=x_cls[n][ds(sh * D, D)],
                                    accum_op=OP.add)
            sc3 = cls_pool.tile([1, D], F32, tag="sc3")
            ss3 = cls_pool.tile([1, 8], F32, tag="ss3")
            nc.vector.tensor_mul(sc3[:, :], clsum[:, :], clsum[:, :])
            nc.vector.tensor_reduce(out=ss3[:, 0:1], in_=sc3[:, :], axis=AX.X, op=OP.add)
            nc.vector.tensor_mul(sc3[:, :], clsum[:, :], w3[:, :])
            nc.vector.tensor_reduce(out=ss3[:, 1:2], in_=sc3[:, :], axis=AX.X, op=OP.add)
            nc.scalar.sqrt(ss3[:, 2:3], ss3[:, 0:1])
            nc.vector.reciprocal(ss3[:, 3:4], ss3[:, 2:3])
            nc.vector.tensor_mul(ss3[:, 4:5], ss3[:, 1:2], ss3[:, 3:4])
            nc.vector.tensor_add(ss3[:, 5:6], ss3[:, 4:5], bh[:, 0:1])
            nc.sync.dma_start(out=c3d_list[n][:, :], in_=ss3[0:1, 5:6])
            c3b = img_pool.tile([128, 1], F32, tag="c3b")
            nc.sync.dma_start(out=c3b[:, :], in_=c3d_list[n][:, :].to_broadcast((128, 1)))
            im["c3b"] = c3b

        # ================= phase 2: main + tail per image =================
        for n in range(per_core):
            im = ims[n]
            qT, sQ, c3b = im["qT"], im["sQ"], im["c3b"]
            for k in range(KC):
                nc.vector.tensor_copy(sQ[2][:, 2, k, W1LOC:W1LOC + 1],
                                      w1s[:, k:k + 1])
                nc.vector.tensor_copy(qT[:, W2BANK, k, W2LOC:W2LOC + 1],
                                      w2s[:, k:k + 1])
            Mc = img_pool.tile([128, MB, 3], F32, tag="Mc")
            p1_all = img_pool.tile([128, MB], F32, tag="p1")
            pscr = pk_pool.tile([128, 2048], F32, tag="pscr")

            for Qi, (base, bws, pkw) in enumerate(QUADS):
                for m in range(MB):
                    mcols = 128 if m < MB - 1 else (W2BANK * 512 + W2LOC + 1) - 128 * (MB - 1)
                    quad = psum_mm.tile([128, 2048], F32, tag="quad")
                    for k in range(KC):
                        for bi, nb in enumerate(bws):
                            nc.tensor.matmul(
                                quad[:mcols, ds(512 * bi, nb)],
                                lhsT=qT[:, m // 4, k, ds(128 * (m % 4), mcols)],
                                rhs=sQ[Qi][:, bi, k, :nb],
                                start=(k == 0), stop=(k == KC - 1),
                            )
                    nc.vector._custom_dve(
                        PACKMAX, out=pscr[:mcols, :pkw],
                        in0=quad[:mcols, :pkw],
                        in1=iof[:mcols, ds(2048 * Qi, pkw)],
                        s0=PK_C0, s1=PK_C1, imm2=PK_C2,
                        accum_out=Mc[:mcols, m, Qi:Qi + 1],
                    )
                    if m == MB - 1:
                        p2c = cls_pool.tile([1, 2048], F32, tag="p2c")
                        nc.vector.tensor_copy(p2c[0:1, :pkw], quad[96:97, :pkw])
                        nc.sync.dma_start(out=p2d_list[n][ds(base, pkw), 0],
                                          in_=p2c[0:1, :pkw])
                    if Qi == 2:
                        nc.vector.tensor_copy(p1_all[:mcols, m:m + 1],
                                              quad[:mcols, P1LOC:P1LOC + 1])

            # ---- combine: reduce quads, decode pack ----
            red = img_pool.tile([128, MB], F32, tag="red")
            nc.vector.tensor_reduce(out=red[:, :], in_=Mc[:, :, :], axis=AX.X,
                                    op=OP.max)
            dec = img_pool.tile([128, 6 * MB], F32, tag="dec")
            t_ = dec[:, 0 * MB:1 * MB]
            r_ = dec[:, 1 * MB:2 * MB]
            d_ = dec[:, 2 * MB:3 * MB]
            ng = dec[:, 3 * MB:4 * MB]
            u_ = dec[:, 4 * MB:5 * MB]
            k_ = dec[:, 5 * MB:6 * MB]
            nc.vector.tensor_scalar_mul(t_, red[:, :], 1.0 / 8192.0)
            nc.scalar.activation(r_, t_, ACTF.Copy, bias=BIG)
            nc.vector.tensor_scalar_add(r_, r_, -BIG)
            nc.vector.tensor_sub(d_, t_, r_)
            nc.vector.tensor_scalar(ng, d_, 0.0, None, op0=OP.is_lt)
            nc.vector.tensor_add(u_, d_, ng)
            nc.vector.tensor_scalar_mul(u_, u_, 8192.0)
            nc.vector.tensor_sub(k_, r_, ng)
            gidxf = img_pool.tile([128, MB], F32, tag="gidxf")
            nc.scalar.activation(gidxf[:, :], u_, ACTF.Copy, bias=8191.0, scale=-1.0)
            gidx = img_pool.tile([128, MB], U32, tag="gidx")
            nc.vector.tensor_copy(gidx[:, :], gidxf[:, :])
            dmin_all = img_pool.tile([128, MB], F32, tag="dmin")
            nc.scalar.activation(dmin_all[:, :], k_, ACTF.Copy, bias=3.0,
                                 scale=-1.0 / 512.0)

            # ---- p2 gather + head ----
            p2g = img_pool.tile([128, MB], F32, tag="p2g")
            for m in range(MB):
                nc.gpsimd.indirect_dma_start(
                    out=p2g[:, m:m + 1], out_offset=None, in_=p2d_list[n][:, :],
                    in_offset=IndirectOffsetOnAxis(ap=gidx[:, m:m + 1], axis=0),
                )
            lg = img_pool.tile([128, MB], F32, tag="lg")
            nc.vector.tensor_add(lg[:, :], p1_all[:, :], p2g[:, :])
            pred = img_pool.tile([128, MB], F32, tag="pred")
            nc.scalar.activation(pred[:, :], lg[:, :], ACTF.Sigmoid, bias=c3b[:, :])
            o0 = img_pool.tile([128, MB], F32, tag="o0")
            nc.vector.tensor_mul(o0[:, :], pred[:, :], dmin_all[:, :])
            for m in range(MB):
                mreal = 128 if m < MB - 1 else TQ - 128 * (MB - 1)
                nc.sync.dma_start(out=out1[n, ds(m * 128, mreal)],
                                  in_=pred[:mreal, m:m + 1])
                nc.sync.dma_start(out=out0[n, ds(m * 128, mreal)],
                                  in_=o0[:mreal, m:m + 1])

    nc.compile()
    return nc


_CACHED = {}


def _get_program(per_core=PER_CORE):
    if per_core not in _CACHED:
        _CACHED[per_core] = build_program(per_core)
    return _CACHED[per_core]


def run(inputs, trace=False, per_core=PER_CORE):
    nc = _get_program(per_core)
    n_cores = N_FULL // per_core
    xq = np.ascontiguousarray(inputs["x_query"], dtype=np.float32)
    xs = np.ascontiguousarray(inputs["x_support"], dtype=np.float32)
    xc = np.ascontiguousarray(inputs["x_support_cls"], dtype=np.float32).reshape(
        N_FULL, S * D
    )
    wh = np.ascontiguousarray(inputs["W_head"], dtype=np.float32).reshape(3 * D, 1)
    bhv = np.ascontiguousarray(inputs["b_head"], dtype=np.float32).reshape(1, 1)
    in_maps = []
    for c in range(n_cores):
        sl = slice(c * per_core, (c + 1) * per_core)
        in_maps.append({
            "x_query": xq[sl], "x_support": xs[sl], "x_support_cls": xc[sl],
            "W_head": wh, "b_head": bhv,
        })
    res = run_bass_kernel_spmd(nc, in_maps, list(range(n_cores)), trace=trace)
    o0 = np.concatenate([res.results[c]["out0"] for c in range(n_cores)], axis=0)
    o1 = np.concatenate([res.results[c]["out1"] for c in range(n_cores)], axis=0)
    o0 = o0.reshape(N_FULL, 1, SIDE, SIDE).astype(np.float32)
    o1 = o1.reshape(N_FULL, 1, SIDE, SIDE).astype(np.float32)
    return (o0, o1), res


def kernel(**inputs):
    (o0, o1), _ = run(inputs, trace=False)
    return o0, o1


# revision 16
# speedup vs baseline: 1.4090x; 1.3680x over previous
"""Trainium2 Bass kernel: FADEv4 retrieval-kNN head (nn_FADEv4_7026566496861).

Math (per image n):
    cls  = l2norm(mean_s(x_support_cls[n]))          # [1,D]
    q    = l2norm(x_query[n])                        # [Tq,D]
    s    = l2norm(x_support[n])                      # [Ts,D]
    sim  = q @ s.T                                   # [Tq,Ts]
    dmin = 1 - max_ts(sim); idx = argmax_ts(sim)
    pred = sigmoid(q@W1 + s[idx]@W2 + cls@W3 + b)
    out0 = (pred*dmin).reshape(N,1,37,37); out1 = pred.reshape(N,1,37,37)

Sharding: data-parallel over N=16 images -> 8 cores x 2 images, no collectives.

Kernel design notes:
  * sim is computed on the PE as qT.T @ sT where qT/sT are [D, T] tiles built
    by a fused normalize-transpose matmul: out = s_chunk.T @ diag(1/||s||).
  * W1 is appended as an extra support column of sT (sim[:,Ts] = q@W1) and
    W2 as an extra query column of qT (sim[Tq,:] = s@W2), so the head dot
    products fall out of the big matmul for free.
  * max/argmax run on the DVE directly from PSUM in 512-wide chunks (max8 +
    max_index); chunk results are combined with a match_replace one-hot
    trick, giving first-occurrence argmax semantics matching jnp.argmin.
  * p2 = s@W2 is staged to DRAM and gathered per query row by indirect DMA.
"""

import os
from contextlib import ExitStack

import numpy as np

import concourse.bass as bass
import concourse.mybir as mybir
import concourse.tile as tile
from concourse import bacc, bass_isa
from concourse.bass import ds, ts, IndirectOffsetOnAxis
from concourse.bass_utils import run_bass_kernel_spmd
from concourse.masks import make_identity

F32 = mybir.dt.float32
BF16 = mybir.dt.bfloat16
F32R = mybir.dt.float32r
U32 = mybir.dt.uint32
AX = mybir.AxisListType
OP = mybir.AluOpType
ACTF = mybir.ActivationFunctionType

N_FULL, TQ, TS, S, D = 16, 1369, 5476, 4, 768
SIDE = 37
KC = D // 128            # 6 contraction chunks
W2COL = 1376             # W2 column padded out to a quarter-aligned partition
TQE = W2COL + 1          # 1377 qT columns (7 zero pads + W2)
TSE = TS + 1             # 5477 sT columns (incl W1)
MB = (TQE + 127) // 128  # 11 M-blocks (last: 97 cols, 89 real queries)
NB = (TSE + 511) // 512  # 11 N-chunks (last: 357 cols, 356 real supports)
NEG = -1.0e30

N_CORES = 8
PER_CORE = N_FULL // N_CORES

MM_DTYPE = {"f32": F32, "bf16": BF16, "f32r": F32R}[os.environ.get("FADE_MM", "bf16")]


def _emit_image(nc, ctx, tc, pools, consts, aps, n, stage=99):
    """Emit one image's full pipeline."""
    (img_pool, spool, scratch, psum_t, psum_mm) = pools
    (ident, ident_mm, c512f, w1s, w2s, w3, bh, ones1) = consts
    (x_query, x_support, x_cls, p2d_list, c3d_list, out0, out1, mm_dtype) = aps

    if stage < 1:
        z0 = scratch.tile([128, MB], F32, tag="z0")
        nc.vector.memset(z0[:, :], 0.25)
        for m in range(MB):
            mreal = 128 if m < MB - 1 else TQ - 128 * (MB - 1)
            nc.sync.dma_start(out=out1[n, ds(m * 128, mreal)], in_=z0[:mreal, m:m+1])
            nc.sync.dma_start(out=out0[n, ds(m * 128, mreal)], in_=z0[:mreal, m:m+1])
        return

    # ---- cls head scalar: c3b = (sum_cls . W3)/||sum_cls|| + b ----
    clsbig = scratch.tile([1, S * D], F32, tag="clsbig")
    nc.sync.dma_start(out=clsbig[:, :], in_=x_cls[n])
    if stage < 1.1:
        z0 = scratch.tile([128, MB], F32, tag="z0")
        nc.vector.memset(z0[:, :], 0.25)
        nc.vector.tensor_copy(z0[0:1, 0:1], clsbig[0:1, 0:1])
        for m in range(MB):
            mreal = 128 if m < MB - 1 else TQ - 128 * (MB - 1)
            nc.sync.dma_start(out=out1[n, ds(m * 128, mreal)], in_=z0[:mreal, m:m+1])
            nc.sync.dma_start(out=out0[n, ds(m * 128, mreal)], in_=z0[:mreal, m:m+1])
        return
    clsum = scratch.tile([1, D], F32, tag="clsum")
    import os as _os2
    _clsmode = _os2.environ.get("FADE_CLSMODE", "full")
    if _clsmode == "tiny":
        nc.vector.tensor_add(clsum[0:1, 0:1], clsbig[0:1, 0:1], clsbig[0:1, D:D + 1])
    elif _clsmode == "one":
        nc.vector.tensor_add(clsum[:, :], clsbig[:, 0:D], clsbig[:, D:2 * D])
    else:
        nc.vector.tensor_add(clsum[:, :], clsbig[:, 0:D], clsbig[:, D:2 * D])
        nc.vector.tensor_add(clsum[:, :], clsum[:, :], clsbig[:, 2 * D:3 * D])
        nc.vector.tensor_add(clsum[:, :], clsum[:, :], clsbig[:, 3 * D:4 * D])
    cls_sum = clsum[0:1, :]
    if stage < 1.2:
        z0 = scratch.tile([128, MB], F32, tag="z0")
        nc.vector.memset(z0[:, :], 0.25)
        nc.vector.tensor_copy(z0[0:1, 0:1], clsum[0:1, 0:1])
        for m in range(MB):
            mreal = 128 if m < MB - 1 else TQ - 128 * (MB - 1)
            nc.sync.dma_start(out=out1[n, ds(m * 128, mreal)], in_=z0[:mreal, m:m+1])
            nc.sync.dma_start(out=out0[n, ds(m * 128, mreal)], in_=z0[:mreal, m:m+1])
        return
    sc3 = scratch.tile([1, D], F32, tag="sc3")
    ss3 = scratch.tile([1, 8], F32, tag="ss3")
    nc.vector.tensor_mul(sc3[:, :], cls_sum, cls_sum)
    nc.vector.tensor_reduce(out=ss3[:, 0:1], in_=sc3[:, :], axis=AX.X, op=OP.add)
    nc.vector.tensor_mul(sc3[:, :], cls_sum, w3[:, :])
    nc.vector.tensor_reduce(out=ss3[:, 1:2], in_=sc3[:, :], axis=AX.X, op=OP.add)
    if stage < 1.3:
        z0 = scratch.tile([128, MB], F32, tag="z0")
        nc.vector.memset(z0[:, :], 0.25)
        nc.vector.tensor_copy(z0[0:1, 0:2], ss3[0:1, 0:2])
        for m in range(MB):
            mreal = 128 if m < MB - 1 else TQ - 128 * (MB - 1)
            nc.sync.dma_start(out=out1[n, ds(m * 128, mreal)], in_=z0[:mreal, m:m+1])
            nc.sync.dma_start(out=out0[n, ds(m * 128, mreal)], in_=z0[:mreal, m:m+1])
        return
    nc.scalar.sqrt(ss3[:, 2:3], ss3[:, 0:1])
    nc.vector.reciprocal(ss3[:, 3:4], ss3[:, 2:3])
    nc.vector.tensor_mul(ss3[:, 4:5], ss3[:, 1:2], ss3[:, 3:4])
    nc.vector.tensor_add(ss3[:, 5:6], ss3[:, 4:5], bh[:, 0:1])
    if stage < 1.4:
        z0 = scratch.tile([128, MB], F32, tag="z0")
        nc.vector.memset(z0[:, :], 0.25)
        nc.vector.tensor_copy(z0[0:1, 0:1], ss3[0:1, 5:6])
        for m in range(MB):
            mreal = 128 if m < MB - 1 else TQ - 128 * (MB - 1)
            nc.sync.dma_start(out=out1[n, ds(m * 128, mreal)], in_=z0[:mreal, m:m+1])
            nc.sync.dma_start(out=out0[n, ds(m * 128, mreal)], in_=z0[:mreal, m:m+1])
        return
    nc.sync.dma_start(out=c3d_list[n][:, :], in_=ss3[0:1, 5:6])
    c3b = img_pool.tile([128, 1], F32, tag="c3b")
    if stage < 1.45:
        nc.vector.memset(c3b[:, :], 0.125)
    else:
        nc.sync.dma_start(out=c3b[:, :], in_=c3d_list[n][:, :].to_broadcast((128, 1)))

    if stage < 1.5:
        z0 = scratch.tile([128, MB], F32, tag="z0")
        nc.vector.tensor_copy(z0[:, 0:1], c3b[:, :])
        for m in range(MB):
            mreal = 128 if m < MB - 1 else TQ - 128 * (MB - 1)
            nc.sync.dma_start(out=out1[n, ds(m * 128, mreal)], in_=z0[:mreal, 0:1])
            nc.sync.dma_start(out=out0[n, ds(m * 128, mreal)], in_=z0[:mreal, 0:1])
        return

    # ---- normalize+transpose helper ----
    def build_T(dst_all, src_dram_row0, tok0, rows, dst_off):
        """dst_all[:, k, dst_off:dst_off+rows] = normalized-transposed rows."""
        raw = scratch.tile([128, D], F32, tag="nt_raw")
        nc.sync.dma_start(out=raw[:rows, :], in_=src_dram_row0[ds(tok0, rows), :])
        sq = scratch.tile([128, D], F32, tag="nt_sq")
        ssn = scratch.tile([128, 4], F32, tag="nt_ss")
        nc.scalar.activation(
            sq[:rows, :], raw[:rows, :], ACTF.Square, accum_out=ssn[:rows, 0:1]
        )
        nc.scalar.sqrt(ssn[:rows, 1:2], ssn[:rows, 0:1])
        nc.vector.reciprocal(ssn[:rows, 2:3], ssn[:rows, 1:2])
        s_nm = scratch.tile([128, D], mm_dtype, tag="nt_nm")
        nc.scalar.mul(s_nm[:rows, :], raw[:rows, :], ssn[:rows, 2:3])
        for g in range(KC // 3):
            pst = psum_t.tile([128, 3, 512], mm_dtype, tag="nt_ps")
            for kk in range(3):
                k = 3 * g + kk
                nc.tensor.transpose(
                    pst[:, kk, :rows], s_nm[:rows, ts(k, 128)],
                    ident_mm[:rows, :rows],
                )
            nc.scalar.copy(
                dst_all[:, 3 * g:3 * g + 3, ds(dst_off, rows)], pst[:, :, :rows]
            )

    # ---- build qT (full [D, TQE] in mm_dtype) ----
    qT = img_pool.tile([128, KC, TQE], mm_dtype, tag="qT", name="qT")
    for c in range(MB):
        tok0 = c * 128
        rows = min(128, TQ - tok0)
        if rows > 0:
            build_T(qT, x_query[n], tok0, rows, tok0)
    nc.vector.memset(qT[:, :, TQ:W2COL], 0)
    for k in range(KC):
        nc.vector.tensor_copy(qT[:, k, W2COL:W2COL + 1], w2s[:, k:k + 1])

    if stage < 2:
        z0 = scratch.tile([128, MB], F32, tag="z0")
        nc.vector.memset(z0[:, :], 0.25)
        for m in range(MB):
            mreal = 128 if m < MB - 1 else TQ - 128 * (MB - 1)
            nc.sync.dma_start(out=out1[n, ds(m * 128, mreal)], in_=z0[:mreal, m:m+1])
            nc.sync.dma_start(out=out0[n, ds(m * 128, mreal)], in_=z0[:mreal, m:m+1])
        return

    # ---- per-image state ----
    Mc8 = img_pool.tile([128, MB, NB, 8], F32, tag="Mc8")
    Ic8 = img_pool.tile([128, MB, NB, 8], U32, tag="Ic8")
    p1 = img_pool.tile([128, MB], F32, tag="p1")

    # ---- main loop: N-chunks outer ----
    for j in range(NB):
        ncols = 512 if j < NB - 1 else TSE - 512 * (NB - 1)   # 357 on last
        nreal = 512 if j < NB - 1 else TS - 512 * (NB - 1)    # 356 on last
        sT = spool.tile([128, KC, 512], mm_dtype, tag="sT", name="sT")
        off = 0
        while off < nreal:
            rows = min(128, nreal - off)
            build_T(sT, x_support[n], 512 * j + off, rows, off)
            off += rows
        if j == NB - 1:
            for k in range(KC):
                nc.vector.tensor_copy(sT[:, k, nreal:nreal + 1], w1s[:, k:k + 1])

        for m in [MB - 1] + list(range(MB - 1)):
            mcols = 128 if m < MB - 1 else TQE - 128 * (MB - 1)   # 97 on last
            bp = psum_mm.tile([128, 512], F32, tag="bigps")
            for k in range(KC):
                nc.tensor.matmul(
                    bp[:mcols, :ncols],
                    lhsT=qT[:, k, ds(m * 128, mcols)],
                    rhs=sT[:, k, :ncols],
                    start=(k == 0), stop=(k == KC - 1),
                )
            nc.vector.max(Mc8[:mcols, m, j, :], bp[:mcols, :nreal])
            nc.vector.max_index(
                Ic8[:mcols, m, j, :], Mc8[:mcols, m, j, :], bp[:mcols, :nreal]
            )
            if j == NB - 1:
                nc.scalar.copy(p1[:mcols, m:m + 1], bp[:mcols, nreal:nreal + 1])
            if m == MB - 1:
                p2c = scratch.tile([128, 512], F32, tag="p2c")
                nc.scalar.copy(p2c[96:97, :nreal], bp[96:97, :nreal])
                nc.sync.dma_start(
                    out=p2d_list[n][ds(512 * j, nreal), 0], in_=p2c[96:97, :nreal]
                )

    if stage < 3:
        z0 = scratch.tile([128, MB], F32, tag="z0")
        nc.vector.tensor_copy(z0[:, :], Mc8[:, :, 0, 0])
        for m in range(MB):
            mreal = 128 if m < MB - 1 else TQ - 128 * (MB - 1)
            nc.sync.dma_start(out=out1[n, ds(m * 128, mreal)], in_=z0[:mreal, m:m+1])
            nc.sync.dma_start(out=out0[n, ds(m * 128, mreal)], in_=p1[:mreal, m:m+1])
        return

    # ---- combine chunk maxima -> gmax / global argmax ----
    gidx_all = img_pool.tile([128, MB], U32, tag="gidx")
    dmin_all = img_pool.tile([128, MB], F32, tag="dmin")
    nc.vector.memset(gidx_all[:, :], 0)
    W = NB * 8
    for m in range(MB):
        mreal = 128 if m < MB - 1 else TQ - 128 * (MB - 1)   # 89 on last
        mc = Mc8[:mreal, m, :, :].rearrange("p a b -> p (a b)")
        ic = Ic8[:mreal, m, :, :].rearrange("p a b -> p (a b)")
        gm8 = scratch.tile([128, 8], F32, tag="gm8")
        nc.vector.max(gm8[:mreal, :], mc)
        mri = scratch.tile([128, 8], F32, tag="mri")
        nc.vector.memset(mri[:mreal, :], NEG)
        nc.vector.tensor_copy(mri[:mreal, 0:1], gm8[:mreal, 0:1])
        mrep = scratch.tile([128, W], F32, tag="mrep")
        nc.vector.match_replace(
            out=mrep[:mreal, :], in_to_replace=mri[:mreal, :],
            in_values=mc, imm_value=NEG,
        )
        oneh = scratch.tile([128, W], F32, tag="oneh")
        nc.vector.tensor_tensor(oneh[:mreal, :], mrep[:mreal, :], mc, OP.not_equal)
        icf = scratch.tile([128, W], F32, tag="icf")
        nc.vector.tensor_copy(icf[:mreal, :], ic)
        nc.vector.tensor_add(icf[:mreal, :], icf[:mreal, :], c512f[:mreal, :])
        scr = scratch.tile([128, W], F32, tag="scr")
        gidxf = scratch.tile([128, 1], F32, tag="gidxf")
        nc.vector.tensor_mul(scr[:mreal, :], oneh[:mreal, :], icf[:mreal, :])
        nc.vector.tensor_reduce(
            out=gidxf[:mreal, :], in_=scr[:mreal, :], axis=AX.X, op=OP.add
        )
        nc.vector.tensor_copy(gidx_all[:mreal, m:m + 1], gidxf[:mreal, :])
        nc.scalar.activation(
            dmin_all[:mreal, m:m + 1], gm8[:mreal, 0:1], ACTF.Copy,
            bias=1.0, scale=-1.0,
        )

    if stage < 4:
        for m in range(MB):
            mreal = 128 if m < MB - 1 else TQ - 128 * (MB - 1)
            nc.sync.dma_start(out=out1[n, ds(m * 128, mreal)], in_=dmin_all[:mreal, m:m+1])
            fgi = scratch.tile([128, 1], F32, tag="fgi")
            nc.vector.tensor_copy(fgi[:, :], gidx_all[:, m:m+1])
            nc.sync.dma_start(out=out0[n, ds(m * 128, mreal)], in_=fgi[:mreal, :])
        return

    # ---- p2 gather + head ----
    p2g = img_pool.tile([128, MB], F32, tag="p2g")
    for m in range(MB):
        nc.gpsimd.indirect_dma_start(
            out=p2g[:, m:m + 1], out_offset=None, in_=p2d_list[n][:, :],
            in_offset=IndirectOffsetOnAxis(ap=gidx_all[:, m:m + 1], axis=0),
        )
    if stage < 4.5:
        for m in range(MB):
            mreal = 128 if m < MB - 1 else TQ - 128 * (MB - 1)
            nc.sync.dma_start(out=out1[n, ds(m * 128, mreal)], in_=p1[:mreal, m:m+1])
            nc.sync.dma_start(out=out0[n, ds(m * 128, mreal)], in_=p2g[:mreal, m:m+1])
        return

    for m in range(MB):
        mreal = 128 if m < MB - 1 else TQ - 128 * (MB - 1)
        lg = scratch.tile([128, 1], F32, tag="lg")
        nc.vector.tensor_add(lg[:mreal, :], p1[:mreal, m:m + 1], p2g[:mreal, m:m + 1])
        pred = scratch.tile([128, 1], F32, tag="pred")
        nc.scalar.activation(
            pred[:mreal, :], lg[:mreal, :], ACTF.Sigmoid, bias=c3b[:mreal, :]
        )
        o0 = scratch.tile([128, 1], F32, tag="o0")
        nc.vector.tensor_mul(o0[:mreal, :], pred[:mreal, :], dmin_all[:mreal, m:m + 1])
        nc.sync.dma_start(out=out1[n, ds(m * 128, mreal)], in_=pred[:mreal, :])
        nc.sync.dma_start(out=out0[n, ds(m * 128, mreal)], in_=o0[:mreal, :])


def build_program(per_core=PER_CORE, mm_dtype=MM_DTYPE, stage=99):
    nc = bacc.Bacc("TRN2", target_bir_lowering=False, debug=False)
    x_query = nc.dram_tensor("x_query", [per_core, TQ, D], F32, kind="ExternalInput").ap()
    x_support = nc.dram_tensor("x_support", [per_core, TS, D], F32, kind="ExternalInput").ap()
    x_cls = nc.dram_tensor("x_support_cls", [per_core, S * D], F32, kind="ExternalInput").ap()
    w_head = nc.dram_tensor("W_head", [3 * D, 1], F32, kind="ExternalInput").ap()
    b_head = nc.dram_tensor("b_head", [1, 1], F32, kind="ExternalInput").ap()
    out0 = nc.dram_tensor("out0", [per_core, TQ], F32, kind="ExternalOutput").ap()
    out1 = nc.dram_tensor("out1", [per_core, TQ], F32, kind="ExternalOutput").ap()
    p2d_list = [
        nc.dram_tensor(f"p2d_{n}", [TS, 1], F32).ap()
        for n in range(per_core)
    ]
    c3d_list = [
        nc.dram_tensor(f"c3d_{n}", [1, 1], F32).ap()
        for n in range(per_core)
    ]

    with tile.TileContext(nc) as tc, ExitStack() as ctx:
        img_pool = ctx.enter_context(tc.tile_pool(name="img", bufs=2))
        spool = ctx.enter_context(tc.tile_pool(name="sT", bufs=3))
        scratch = ctx.enter_context(tc.tile_pool(name="scratch", bufs=3))
        const_pool = ctx.enter_context(tc.tile_pool(name="const", bufs=1))
        psum_t = ctx.enter_context(tc.tile_pool(name="psum_t", bufs=2, space="PSUM"))
        psum_mm = ctx.enter_context(tc.tile_pool(name="psum_mm", bufs=4, space="PSUM"))

        # constants
        import os as _os
        _noconst = _os.environ.get("FADE_NOCONST", "0") == "1"
        ident = const_pool.tile([128, 128], F32)
        ident_mm = const_pool.tile([128, 128], mm_dtype)
        if _noconst:
            nc.vector.memset(ident[:, :], 0.0)
            nc.vector.memset(ident_mm[:, :], 0.0)
        else:
            make_identity(nc, ident[:, :])
            make_identity(nc, ident_mm[:, :])
        c512u = const_pool.tile([128, NB * 8], U32)
        c512f = const_pool.tile([128, NB * 8], F32)
        if _noconst:
            nc.vector.memset(c512f[:, :], 0.0)
        else:
            nc.gpsimd.iota(c512u[:, :], pattern=[[512, NB], [0, 8]], base=0,
                           channel_multiplier=0)
            nc.vector.tensor_copy(c512f[:, :], c512u[:, :])
        ones1 = const_pool.tile([1, 128], F32)
        nc.vector.memset(ones1[:, :], 1.0)
        w1s = const_pool.tile([128, KC], F32)
        w2s = const_pool.tile([128, KC], F32)
        w3 = const_pool.tile([1, D], F32)
        bh = const_pool.tile([1, 1], F32)
        for k in range(KC):
            nc.sync.dma_start(out=w1s[:, k:k + 1], in_=w_head[ds(128 * k, 128), :])
            nc.sync.dma_start(out=w2s[:, k:k + 1], in_=w_head[ds(D + 128 * k, 128), :])
        nc.sync.dma_start(out=w3[0:1, :], in_=w_head[ds(2 * D, D), :])
        nc.sync.dma_start(out=bh[:, :], in_=b_head[:, :])

        pools = (img_pool, spool, scratch, psum_t, psum_mm)
        consts = (ident, ident_mm, c512f, w1s, w2s, w3, bh, ones1)
        aps = (x_query, x_support, x_cls, p2d_list, c3d_list, out0, out1, mm_dtype)
        for n in range(per_core):
            _emit_image(nc, ctx, tc, pools, consts, aps, n, stage=stage)

    nc.compile()
    return nc


_CACHED = {}


def _get_program(per_core=PER_CORE, mm_dtype=MM_DTYPE):
    key = (per_core, mm_dtype)
    if key not in _CACHED:
        _CACHED[key] = build_program(per_core, mm_dtype)
    return _CACHED[key]


def run(inputs, trace=False, per_core=PER_CORE, mm_dtype=MM_DTYPE):
    nc = _get_program(per_core, mm_dtype)
    n_cores = N_FULL // per_core
    xq = np.ascontiguousarray(inputs["x_query"], dtype=np.float32)
    xs = np.ascontiguousarray(inputs["x_support"], dtype=np.float32)
    xc = np.ascontiguousarray(inputs["x_support_cls"], dtype=np.float32).reshape(
        N_FULL, S * D
    )
    wh = np.ascontiguousarray(inputs["W_head"], dtype=np.float32).reshape(3 * D, 1)
    bhv = np.ascontiguousarray(inputs["b_head"], dtype=np.float32).reshape(1, 1)
    in_maps = []
    for c in range(n_cores):
        sl = slice(c * per_core, (c + 1) * per_core)
        in_maps.append({
            "x_query": xq[sl], "x_support": xs[sl], "x_support_cls": xc[sl],
            "W_head": wh, "b_head": bhv,
        })
    res = run_bass_kernel_spmd(nc, in_maps, list(range(n_cores)), trace=trace)
    o0 = np.concatenate([res.results[c]["out0"] for c in range(n_cores)], axis=0)
    o1 = np.concatenate([res.results[c]["out1"] for c in range(n_cores)], axis=0)
    o0 = o0.reshape(N_FULL, 1, SIDE, SIDE).astype(np.float32)
    o1 = o1.reshape(N_FULL, 1, SIDE, SIDE).astype(np.float32)
    return (o0, o1), res


def kernel(**inputs):
    (o0, o1), _ = run(inputs, trace=False)
    return o0, o1

